# revision 43
# baseline (speedup 1.0000x reference)
"""AtomGMMProjector Bass kernel for Trainium2 (8 NeuronCores, SPMD).

Math (per batch b):
    cx = centers @ R[b,0], cy = centers @ R[b,1]          (rotated atom x/y)
    z_{x,y}[n,d] = (line[d] - c[n]) / (sqrt(2)*sigma[n])  (scaled distance)
    proj[y,x]    = sum_n amp[n] * exp(-zy[n,y]^2) * exp(-zx[n,x]^2)

Sharding: data-parallel over batch B=32 -> 4 batches per core, 8 cores.

2D binning: atoms are host-sorted by rotated y into bins (bin boundaries at
128-atom tile granularity, chosen by a small DP with HW-calibrated weights:
per-matmul fixed cost dominates on HW, columns are cheap), then by rotated
x within each bin.  Each bin gets a 32-aligned y-column window (matmul
output partition bases are restricted to the PE's 32-row grid), each tile
an 8-aligned x-column window (8-alignment matters: 4-byte-aligned SBUF
operands take a measured ~5-9us/body slow path).  Windows are unions over
all 32 batches (one SPMD program for every core) of per-atom
c +/- MARGIN_SIG*sigma footprints.

Default pipeline (QUAD_EXP + QUAD_FUSED), per 128-atom tile (atom ->
partition; measured per-op costs on this stack: DVE tensor_scalar ~150 ns,
ACT activation cheap when batched, PE matmul ~payload-only, GPSIMD ~3.4 us
-> the design minimizes per-tile ops and keeps them on PE/ACT):
    v   = one contract-22 bf16 PE matmul into PSUM computing BOTH window
          segments [y | x]: v_y = -zy^2 and v_x = -zx^2 + ln(amp).  lhsT
          stacks 11 x-rows and 11 y-rows of triple-bf16-split coefficients
          of the quadratic -s^2*l'^2 + 2sb*l' - b^2 (+ln amp) in per-bin
          shifted line coords l'; the host-baked rhs [22, yw+xw] holds
          [l'^2(hi,lo) l' 1] rows, x-rows zeroed on the y segment and vice
          versa.  Triple splitting keeps |err(v)| < ~0.01 where it matters.
    e   = Exp(v) on ACT, batched over groups of tiles (one PSUM->SBUF bf16
          op per ~1024-col group; bank-padding gaps computed and ignored).
    out += e_y.T @ e_x : accumulating bf16 PE matmuls into the y-windowed,
          partition-sliced PSUM accumulator (pieces split at the 128-row
          boundary / 32-row tile-position grid).
    sigma is clamped to >= QUAD_SIG_MIN for the squared form (sub-pixel
    atoms; clamp error far below tolerance on both input regimes).

Everything else is host-precomputed and DMA'd: no on-device coefficient
chains.  PSUM accumulators are DVE-memset-zeroed and the matmuls accumulate
with start=False (start=True only marks a pending-zero region consumed by
PE first-touch writes, so never-written columns would read stale).  Output
copies run on DVE as bf16 into the row/col union bounding box only -- the
DRAM output is zero-donated -- and the host converts back to f32.

Scheduling (see _body_pq): the repeat x batch x group loops are flattened
into one software-pipelined stream; z-matmuls of unit i+1 are issued before
the accumulation matmuls of unit i so the exp stream never waits on PE at
group/batch boundaries.  z-pairs are packed by randomized-restart first-fit
into <=512-col PSUM banks (a single matmul output may NOT span banks --
walrus rejects it), then LPT-bucketed into balanced exp groups whose rqf2
columns stay contiguous so the const DMA is chunked per group.  Coef DMAs
prefetch one batch ahead; PSUM output accumulators are zeroed lazily.

Older variants (DVE z-build + Derivative_Erf; unfused 4-row PE z-build)
remain selectable via flags for benching: see Z_VIA_PE / QUAD_EXP /
QUAD_FUSED.
"""

import os

import numpy as np
from contextlib import ExitStack

import ml_dtypes

import concourse.bass as bass
import concourse.bacc as bacc
import concourse.mybir as mybir
import concourse.tile as tile
from concourse.bass_utils import run_bass_kernel_spmd

F32 = mybir.dt.float32
BF16 = mybir.dt.bfloat16
AF = mybir.ActivationFunctionType
OP = mybir.AluOpType

B, N, D = 32, 4096, 256
NCORES = 8
BPC = B // NCORES          # batches per core
NT = N // 128              # 128-atom tiles
MARGIN_SIG = float(os.environ.get("KMARGIN", "2.75"))
                           # window margin in units of each atom's sigma
MAXBIN = 10                # max tiles per y-bin in the DP
GROUP_COLS = 3072          # target z/erf' group width (columns)
FIRST_GROUP_COLS = 1024    # shorter first group -> shorter pipeline fill
PI_4 = 0.7853981633974483
INV_SQRT2 = 0.7071067811865476
# DP cost weights, HW-calibrated (ns): per-matmul fixed cost (ldweights +
# issue), per-column PE z/exp stream cost, per-column mm stream cost
W_MM, W_COL, W_XCOL = 118.0, 0.6, 0.42

OUT_COPY = os.environ.get("KOUTCOPY", "dve")
                           # "act" | "act_dve" | "dve" | "gpsimd" | "psum"
                           # ("psum": DMA the output straight from PSUM, f32)
OUT_BF16 = OUT_COPY != "psum"
                           # write the output as bf16 (host converts back to
                           # f32); halves output DMA traffic
ZERO_ENG = "dve"           # "pe": start=True zero matmul; "dve": memset;
                           # "act": Copy with scale=0 off the line const
AMP_MIN_SIDE = True        # apply amp on the narrower of the y/x windows
AY_ENG = "vector"          # amp-multiply engine: "vector"|"scalar"|"gpsimd"
ZX_ENG = "vector"          # x-side z-build engine: "vector"|"gpsimd"
Z_VIA_PE = False           # build z on the PE (contract-4 matmuls with
                           # hi/lo-split bf16 coefficients into PSUM) and
                           # erf' reads PSUM; frees all DVE z ops
ZPS_COLS = 1536            # z-PSUM group tile (banks per buf)
ERF_RUNS = False           # erf' per contiguous run (True) or one op per
                           # group spanning bank-padding gaps (False; gaps
                           # hold stale-but-finite z values, outputs unused)
QUAD_EXP = True            # PE computes v = -z^2 (+ ln amp' on the x side)
                           # directly via an 11-row hi/lo-split contraction
                           # against per-bin rhs rows [l'^2, l', 1]; ACT runs
                           # Exp(v) so the x side comes out amp-scaled and
                           # the per-tile DVE amp op disappears.  Implies
                           # Z_VIA_PE.  sigma is clamped to >= QUAD_SIG_MIN
                           # (the squared form loses the tiny-sigma regime
                           # to cancellation; such atoms are sub-pixel and
                           # the clamp error is far below tolerance).
QUAD_SIG_MIN = 0.05
QUAD_FUSED = True          # one contract-22 z-matmul per tile: lhsT stacks
                           # the x/y coefficient rows, rhs is a host-baked
                           # [22, yw+xw] block (y rows zero at x cols and
                           # vice versa); halves PE z-op count
ZFUSE_K = 5                # max tiles per fused z-matmul (22*K <= 128)
ZBANK = 512                # max z-matmul output cols (1 PSUM bank; raise to
                           # test whether a single matmul may span banks)
QUAD_PAIR = True           # fuse GROUPS of tiles into one contract-22*k
                           # z-matmul (rows 0:22 = tile A active on A's
                           # cols, 22:44 = tile B on B's); pair segment
                           # must fit one PSUM bank (<=512 f32)
SKIP = set(filter(None, os.environ.get("KSKIP", "").split(",")))
                           # bench bisection: subset of {z,exp,amp,mm,out}


# ---------------------------------------------------------------------------
# Host planning: y-bins via DP, per-tile x windows, packed coefficients.
# ---------------------------------------------------------------------------

XALIGN = int(os.environ.get("KXALIGN", "8"))


def _win(lo, hi, base, step, margin, align=None):
    if align is None:
        align = XALIGN
    """Padded grid-column window [w0, w1) covering [lo-margin, hi+margin].
    y windows use align=32: matmul output partition bases are restricted to
    the PE array's 32-row tile grid."""
    l = (lo - margin - base) / step
    h = (hi + margin - base) / step
    w0 = int(np.floor(l / align) * align)
    w1 = int(np.ceil((h + 1.0) / align) * align)
    w0 = max(0, min(D, w0))
    w1 = max(0, min(D, w1))
    return (w0, w1) if w1 > w0 else (0, 0)


def _mm_pieces(s, e):
    """Decompose 32-aligned output rows [s, e) into valid PE matmul pieces
    (h, base, w): base 0/32/64/96 for w<=32, 0/64 for w<=64, 0 for w<=128."""
    out = []
    for h, lo, hi in ((0, s, min(e, 128)), (1, max(s, 128), e)):
        if hi <= lo:
            continue
        b, w = lo - 128 * h, hi - lo
        if w <= 32 or (w <= 64 and b in (0, 64)) or b == 0:
            out.append((h, b, w))
        else:  # b in {32, 96}: peel a 32-row piece, remainder is then valid
            out.append((h, b, 32))
            out.append((h, b + 32, w - 32))
    return out


def plan(line, rot, centers, sigmas):
    """Choose y-bins (tile-granular, DP) + per-tile x windows; return the
    per-batch atom order and the bin structure shared by all batches."""
    step = float(line[1] - line[0])
    base = float(line[0])
    sig = np.maximum(sigmas, 1e-5)
    foot = MARGIN_SIG * sig               # per-atom footprint half-width
    cx = np.einsum("bj,bnj->bn", rot[:, 0, :], centers)
    cy = np.einsum("bj,bnj->bn", rot[:, 1, :], centers)
    ordy = np.argsort(cy, axis=1)
    cys = np.take_along_axis(cy, ordy, axis=1)   # (B, N) ascending per batch
    cxs = np.take_along_axis(cx, ordy, axis=1)   # cx in cy-rank order
    fy = foot[ordy]                              # footprint in cy-rank order

    # candidate bin [i, j): y window + per-tile x windows (unions over B,
    # window edges from per-atom c +/- MARGIN_SIG*sigma footprints)
    def bin_windows(i, j):
        sl = slice(128 * i, 128 * j)
        yw = _win((cys[:, sl] - fy[:, sl]).min(),
                  (cys[:, sl] + fy[:, sl]).max(), base, step, 0.0, align=32)
        ox = np.argsort(cxs[:, sl], axis=1)
        seg = np.take_along_axis(cxs[:, sl], ox, axis=1)
        sgf = np.take_along_axis(fy[:, sl], ox, axis=1)
        xws = []
        for t in range(j - i):
            tl = slice(128 * t, 128 * (t + 1))
            xws.append(_win((seg[:, tl] - sgf[:, tl]).min(),
                            (seg[:, tl] + sgf[:, tl]).max(),
                            base, step, 0.0))
        return yw, xws

    cache = {}
    def cost(i, j):
        yw, xws = cache.setdefault((i, j), bin_windows(i, j))
        if yw[1] <= yw[0]:
            return 0.0
        n_mm = len(_mm_pieces(yw[0], yw[1]))
        c = 0.0
        for (x0, x1) in xws:
            if x1 > x0:
                c += (W_MM * n_mm + W_COL * (yw[1] - yw[0] + x1 - x0)
                      + W_XCOL * n_mm * (x1 - x0))
        return c

    best = [0.0] + [np.inf] * NT
    prev = [0] * (NT + 1)
    for j in range(1, NT + 1):
        for i in range(max(0, j - MAXBIN), j):
            c = best[i] + cost(i, j)
            if c < best[j]:
                best[j] = c
                prev[j] = i
    cuts = []
    j = NT
    while j > 0:
        i = prev[j]
        cuts.append((i, j))
        j = i
    cuts.reverse()

    bins = []
    for (i, j) in cuts:
        yw, xws = cache[(i, j)]
        live_x = [x for x in xws if x[1] > x[0]]
        x_lo = min((x[0] for x in live_x), default=0)
        x_hi = max((x[1] for x in live_x), default=0)
        # per-bin coordinate shifts for the QUAD_EXP rhs rows, rounded so
        # shifted line coords stay bf16-exact for integer grids
        bins.append({"t0": i, "t1": j, "yw": yw, "xws": xws,
                     "sx": round(base + step * 0.5 * (x_lo + x_hi - 1)),
                     "sy": round(base + step * 0.5 * (yw[0] + yw[1] - 1))})

    # final per-batch atom order: by y-bin, then cx within bin
    order = np.empty((B, N), dtype=np.int64)
    for b in range(B):
        pos = 0
        for (i, j) in cuts:
            idx = ordy[b, 128 * i:128 * j]
            idx = idx[np.argsort(cx[b, idx], kind="stable")]
            order[b, pos:pos + len(idx)] = idx
            pos += len(idx)
    return order, bins, cx, cy


def _tiles(v):
    """(N,) per-atom array -> [128, NT] tile layout (tile a = col a)."""
    return np.ascontiguousarray(v.reshape(NT, 128).T)


def _hilo(v):
    hi = v.astype(ml_dtypes.bfloat16)
    lo = (v - hi.astype(np.float32)).astype(ml_dtypes.bfloat16)
    return hi, lo


def _split3(v):
    """Triple bf16 split of a float64 array (combined rel err ~2^-24)."""
    c0 = v.astype(ml_dtypes.bfloat16)
    r = v - c0.astype(np.float64)
    c1 = r.astype(ml_dtypes.bfloat16)
    c2 = (r - c1.astype(np.float64)).astype(ml_dtypes.bfloat16)
    return c0, c1, c2


def _quad_lhs(s, c, shift, q):
    """11 bf16 lhsT rows per atom for v(l) = -(s*(l-shift) - b)^2 + q,
    b = s*(c-shift), matching the rhs row order
    [L0,L0,L0,L1,L1, l',l',l', 1,1,1] with L = l'^2 = L0+L1."""
    b = s * (c - shift)
    A0, A1, A2 = _split3(-s * s)
    B0, B1, B2 = _split3(2.0 * s * b)
    C0, C1, C2 = _split3(-b * b + q)
    return np.stack([A0, A1, A2, A0, A1, B0, B1, B2, C0, C1, C2])


def _quad_rhs(line, shift):
    """11 bf16 rhs rows for the quad contraction at one bin shift."""
    lp = (line.astype(np.float64) - shift)
    L = lp * lp
    L0 = L.astype(ml_dtypes.bfloat16)
    L1 = (L - L0.astype(np.float64)).astype(ml_dtypes.bfloat16)
    lpb = lp.astype(ml_dtypes.bfloat16)
    one = np.ones_like(lpb)
    return np.stack([L0, L0, L0, L1, L1, lpb, lpb, lpb, one, one, one])


def make_in_maps(line_coords, rot_mats, centers, sigmas, amplitudes):
    order, bins, cx, cy = plan(line_coords, rot_mats, centers, sigmas)
    sig = np.maximum(sigmas, 1e-5)
    spos = (INV_SQRT2 / sig).astype(np.float32)
    ampp = (amplitudes * PI_4).astype(np.float32)
    # quad-path per-atom values (float64; sigma clamped, see QUAD_EXP)
    sq = INV_SQRT2 / np.maximum(sigmas.astype(np.float64), QUAD_SIG_MIN)
    # no pi/4 here: the quad path computes exp() directly, with no
    # Derivative_Erf (2/sqrt(pi))^2 factor to undo
    lnamp = np.log(np.maximum(amplitudes.astype(np.float64), 1e-30))
    # bin id per sorted atom position (tile-granular)
    binof = np.zeros(NT, dtype=np.int64)
    for kb, bn in enumerate(bins):
        binof[bn["t0"]:bn["t1"]] = kb
    sx = np.array([bn["sx"] for bn in bins])
    sy = np.array([bn["sy"] for bn in bins])
    shx = np.repeat(sx[binof], 128)   # (N,) per sorted position
    shy = np.repeat(sy[binof], 128)
    coefs = np.empty((B, 128, 4 * NT), dtype=np.float32)
    coefx = np.empty((B, 4, N), dtype=ml_dtypes.bfloat16)
    coefy = np.empty((B, 4, N), dtype=ml_dtypes.bfloat16)
    coefqx = np.empty((B, 11, N), dtype=ml_dtypes.bfloat16)
    coefqy = np.empty((B, 11, N), dtype=ml_dtypes.bfloat16)
    for b in range(B):
        idx = order[b]
        sp = spos[idx]
        bnx = (cx[b, idx] * sp).astype(np.float32)
        bny = (cy[b, idx] * sp).astype(np.float32)
        coefs[b, :, 0 * NT:1 * NT] = _tiles(sp)
        coefs[b, :, 1 * NT:2 * NT] = _tiles(bnx)
        coefs[b, :, 2 * NT:3 * NT] = _tiles(bny)
        coefs[b, :, 3 * NT:4 * NT] = _tiles(ampp[idx])
        coefx[b, 0], coefx[b, 1] = _hilo(sp)
        coefx[b, 2], coefx[b, 3] = _hilo(bnx)
        coefy[b, 0], coefy[b, 1] = _hilo(sp)
        coefy[b, 2], coefy[b, 3] = _hilo(bny)
        sqi = sq[idx]
        coefqx[b] = _quad_lhs(sqi, cx[b, idx].astype(np.float64), shx,
                              lnamp[idx])
        coefqy[b] = _quad_lhs(sqi, cy[b, idx].astype(np.float64), shy, 0.0)
    line_bc = np.ascontiguousarray(
        np.broadcast_to(line_coords.astype(ml_dtypes.bfloat16), (128, D)))
    r4 = np.zeros((4, D), dtype=ml_dtypes.bfloat16)
    r4[0] = line_coords.astype(ml_dtypes.bfloat16)
    r4[1] = r4[0]
    r4[2] = -1.0
    r4[3] = -1.0
    nb = len(bins)
    rq = np.zeros((max(nb, 1), 2, 11, D), dtype=ml_dtypes.bfloat16)
    for kb, bn in enumerate(bins):
        rq[kb, 0] = _quad_rhs(line_coords, bn["sx"])
        rq[kb, 1] = _quad_rhs(line_coords, bn["sy"])
    # fused rhs: per tile [22, yw+xw] block; rows 0:11 (x coefs) are zero on
    # the y segment and rows 11:22 (y coefs) zero on the x segment
    entries = _emit_tiles(bins)
    tot = max(sum((e["y1"] - e["y0"]) + (e["x1"] - e["x0"])
                  for e in entries), 1)
    rqf = np.zeros((22, tot), dtype=ml_dtypes.bfloat16)
    for e in entries:
        yw = e["y1"] - e["y0"]
        xw = e["x1"] - e["x0"]
        o = e["roff"]
        rqf[11:22, o:o + yw] = rq[e["bin"], 1][:, e["y0"]:e["y1"]]
        rqf[0:11, o + yw:o + yw + xw] = rq[e["bin"], 0][:, e["x0"]:e["x1"]]
    coefq = np.concatenate([coefqx, coefqy], axis=1)  # [B, 22, N]
    # pair-fused variants: two tiles per contract-44 z-matmul
    pairs = _zpairs(entries)
    npair = max(len(pairs), 1)
    rqf2 = np.zeros((22 * ZFUSE_K, tot), dtype=ml_dtypes.bfloat16)
    coefq2 = np.zeros((B, 22 * ZFUSE_K, 128 * npair),
                      dtype=ml_dtypes.bfloat16)
    for j, pr in enumerate(pairs):
        o = pr[0]["poff"]
        for i, e in enumerate(pr):
            yw = e["y1"] - e["y0"]
            xw = e["x1"] - e["x0"]
            r0 = 22 * i
            rqf2[r0 + 11:r0 + 22, o:o + yw] = rq[e["bin"], 1][:, e["y0"]:e["y1"]]
            rqf2[r0:r0 + 11, o + yw:o + yw + xw] = \
                rq[e["bin"], 0][:, e["x0"]:e["x1"]]
            a = e["a"]
            coefq2[:, r0:r0 + 22, 128 * j:128 * (j + 1)] = \
                coefq[:, :, 128 * a:128 * (a + 1)]
            o += yw + xw
    in_maps = []
    for c in range(NCORES):
        s = slice(c * BPC, (c + 1) * BPC)
        in_maps.append({
            "line_bc": line_bc,
            "coefs": np.ascontiguousarray(coefs[s]),
            "coefx": np.ascontiguousarray(coefx[s]),
            "coefy": np.ascontiguousarray(coefy[s]),
            "coefqx": np.ascontiguousarray(coefqx[s]),
            "coefqy": np.ascontiguousarray(coefqy[s]),
            "coefq": np.ascontiguousarray(coefq[s]),
            "coefq2": np.ascontiguousarray(coefq2[s]),
            "r4": r4,
            "rq": rq,
            "rqf": rqf,
            "rqf2": rqf2,
        })
    return bins, in_maps


# ---------------------------------------------------------------------------
# Device program.
# ---------------------------------------------------------------------------

def _emit_tiles(bins):
    """Flatten bins into per-tile work entries; cull empty windows.
    roff: per-tile column offset into the fused rhs const."""
    out = []
    roff = 0
    for kb, bn in enumerate(bins):
        y0, y1 = bn["yw"]
        if y1 <= y0:
            continue
        for t, (x0, x1) in enumerate(bn["xws"]):
            if x1 <= x0:
                continue
            out.append({"a": bn["t0"] + t, "y0": y0, "y1": y1,
                        "x0": x0, "x1": x1, "bin": kb, "roff": roff})
            roff += (y1 - y0) + (x1 - x0)
    return out


def _zpairs(entries):
    """Pack live tiles into pairs whose combined [y|x] segments fit one
    PSUM bank (<=512 cols): FFD plus randomized-restart first-fit, aiming
    for ceil(tot/512) pairs -- fewer pairs means fewer exp groups (fewer
    ACT bubbles) and less bank padding."""
    def pack(order):
        prs, space = [], []
        for i in order:
            e = entries[i]
            we = (e["y1"] - e["y0"]) + (e["x1"] - e["x0"])
            for j, sp in enumerate(space):
                if we <= sp and len(prs[j]) < ZFUSE_K:
                    prs[j].append(e)
                    space[j] = sp - we
                    break
            else:
                prs.append([e])
                space.append(ZBANK - we)
        return prs

    w = [(e["y1"] - e["y0"]) + (e["x1"] - e["x0"]) for e in entries]
    lower = max(1, -(-sum(w) // ZBANK))
    best = pack(sorted(range(len(entries)), key=lambda i: -w[i]))
    if len(best) > lower:
        rng = np.random.default_rng(0)
        for _ in range(400):
            cand = pack(list(rng.permutation(len(entries))))
            if len(cand) < len(best):
                best = cand
                if len(best) <= lower:
                    break
    # order pairs by balanced (LPT) group assignment so each exp group's
    # pairs -- and their rqf2 columns -- are contiguous; _groups_pe then
    # groups consecutive runs and the group const DMA can be chunked.
    # Placement width: bank-rounded when pairs may not span banks.
    def eff(pr):
        pw = sum((e["y1"] - e["y0"]) + (e["x1"] - e["x0"]) for e in pr)
        return ZBANK * -(-pw // ZBANK) if ZBANK <= 512 else pw

    ngr = max(1, -(-sum(eff(p) for p in best) // ZPS_COLS))
    while True:
        buckets = [[] for _ in range(ngr)]
        loads = [0.0] * ngr
        ok = True
        for pr in sorted(best, key=lambda p: -eff(p)):
            cands = [j for j in range(ngr)
                     if loads[j] + eff(pr) <= ZPS_COLS]
            if not cands:
                ok = False
                break
            j = min(cands, key=lambda j: loads[j])
            buckets[j].append(pr)
            loads[j] += eff(pr)
        if ok:
            break
        ngr += 1
    pairs = []
    for j, bk in enumerate(buckets):
        for pr in bk:
            for e in pr:
                e["grp"] = j
            pairs.append(pr)
    poff = 0
    for j, pr in enumerate(pairs):
        pw = sum((e["y1"] - e["y0"]) + (e["x1"] - e["x0"]) for e in pr)
        for i, e in enumerate(pr):
            e["pj"], e["poff"], e["pw"] = j, poff, pw
            e["plen"] = len(pr)
            e["lead"] = i == 0
        poff += pw
    return pairs


def _groups(entries):
    """Group tiles for batched erf' ops, assigning group-buffer offsets.
    Returns (entries-with-offsets, ext, runs)."""
    if Z_VIA_PE or QUAD_EXP:
        return _groups_pe(entries)
    groups, cur, ext = [], [], 0
    budget = FIRST_GROUP_COLS
    for e in entries:
        w = (e["y1"] - e["y0"]) + (e["x1"] - e["x0"])
        if cur and ext + w > budget:
            groups.append((cur, ext, [(0, ext)]))
            cur, ext = [], 0
            budget = GROUP_COLS
        e = dict(e)
        e["oy"] = ext
        e["ox"] = ext + (e["y1"] - e["y0"])
        cur.append(e)
        ext += w
    if cur:
        groups.append((cur, ext, [(0, ext)]))
    return groups


def _groups_pe(entries):
    """Bank-aware grouping for the PE z-build: segments live in a PSUM
    group tile and must not cross 512-f32 bank boundaries.  `runs` are the
    contiguous used intervals (erf' is issued per run, skipping gaps)."""
    BANK = ZBANK
    groups = []
    cur, ext, runs = [], 0, []

    def placed(start, w):
        if BANK <= 512 and start // BANK != (start + w - 1) // BANK:
            start = (start // BANK + 1) * BANK
        return start, start + w

    def close():
        nonlocal cur, ext, runs
        if cur:
            groups.append((cur, ext, list(runs)))
        cur, ext, runs = [], 0, []

    if QUAD_EXP and QUAD_FUSED and QUAD_PAIR:
        # groups = the balanced buckets _zpairs baked into pair order (the
        # "grp" field): every group's exp is longer than any single group's
        # z-matmul work, and each group's rqf2 columns are contiguous
        prs = _zpairs(entries)
        lastg = None
        for pr in prs:
            if lastg is not None and pr[0]["grp"] != lastg:
                close()
            lastg = pr[0]["grp"]
            pw = pr[0]["pw"]
            s, e2 = placed(ext, pw)
            if runs and runs[-1][1] == s:
                runs[-1] = (runs[-1][0], e2)
            else:
                runs.append((s, e2))
            o = s
            for e in pr:
                e = dict(e)
                yw = e["y1"] - e["y0"]
                xw = e["x1"] - e["x0"]
                e["oy"], e["ox"] = o, o + yw
                o += yw + xw
                cur.append(e)
            ext = e2
        close()
        return groups

    for e in entries:
        yw = e["y1"] - e["y0"]
        xw = e["x1"] - e["x0"]
        if QUAD_EXP and QUAD_FUSED:
            # single fused [y|x] segment per tile
            oy, e2 = placed(ext, yw + xw)
            if cur and e2 > ZPS_COLS:
                close()
                oy, e2 = placed(0, yw + xw)
            ox = oy + yw
            segs = ((oy, e2),)
        else:
            oy, e1 = placed(ext, yw)
            ox, e2 = placed(e1, xw)
            if cur and e2 > ZPS_COLS:
                close()
                oy, e1 = placed(0, yw)
                ox, e2 = placed(e1, xw)
            segs = ((oy, oy + yw), (ox, ox + xw))
        e = dict(e)
        e["oy"], e["ox"] = oy, ox
        for s, t in segs:
            if runs and runs[-1][1] == s:
                runs[-1] = (runs[-1][0], t)
            else:
                runs.append((s, t))
        cur.append(e)
        ext = e2
    close()
    return groups


def _half_info(entries):
    """Per PSUM half: (row0, row1, col0, col1) bounding box of touched
    output cells (row0/row1 are 32-aligned since y windows are), or None."""
    info = [None, None]
    for h in range(2):
        lo, hi = 128 * h, 128 * (h + 1)
        es = [e for e in entries if e["y0"] < hi and e["y1"] > lo]
        if not es:
            continue
        r0 = min(max(e["y0"] - lo, 0) for e in es)
        r1 = max(min(e["y1"] - lo, 128) for e in es)
        c0 = min(e["x0"] for e in es)
        c1 = max(e["x1"] for e in es)
        info[h] = (r0, r1, c0, c1)
    return info


def _body_pq(nc, pools, lb, r4t, coefx, out, bins, repeats=1):
    """Pipelined QUAD_EXP+QUAD_FUSED+QUAD_PAIR body: the repeat x batch x
    group loops are flattened into one work stream and the z-matmuls of
    unit i+1 are issued before the accumulation matmuls of unit i, so
    ACT's exp stream never waits on PE at group/batch/body boundaries."""
    entries = _emit_tiles(bins)
    for k, e in enumerate(entries):
        e["k"] = k
    last_h = [None, None]
    for k, e in enumerate(entries):
        if e["y0"] < 128:
            last_h[0] = k
        if e["y1"] > 128:
            last_h[1] = k
    halves = _half_info(entries)
    groups = _groups(entries)
    coefp, ep = pools["coef"], pools["e"]
    osbp, psp = pools["osb"], pools["ps"]

    # coef DMAs are prefetched one batch ahead; ps zeroing is lazy (the ps
    # pool may have fewer bufs than BPC -- zero at first use so the DVE
    # memset is issued after the buffer's previous reader)
    state = {}   # (rep, b) -> [cxt, ps]

    def ensure_coef(r, b):
        st = state.setdefault((r, b), [None, None])
        if st[0] is None:
            cxt = coefp.tile(list(coefx[b].shape), BF16, tag="cx",
                             name="cxt")
            nc.sync.dma_start(cxt[:], coefx[b])
            st[0] = cxt
        return st[0]

    def issue_zero(r, b):
        pst = psp.tile([128, 2 * D], F32, tag="ps", name="ps")
        ps = [pst[:, 0:D], pst[:, D:2 * D]]
        for h in range(2):
            if halves[h] is None:
                continue
            r0, r1, c0, c1 = halves[h]
            nc.vector.memset(ps[h][r0:r1, c0:c1], 0.0)
        state[(r, b)][1] = ps

    ng = len(groups)
    nun = repeats * BPC * ng

    def issue_z(i):
        rb, gi = divmod(i, ng)
        r, b = divmod(rb, BPC)
        cxt = ensure_coef(r, b)
        if gi == 0:
            issue_zero(r, b)
            if rb + 1 < repeats * BPC:
                ensure_coef(*divmod(rb + 1, BPC))
        grp, ext, runs = groups[gi]
        zps = pools["zps"].tile([128, ZPS_COLS], F32, tag="zps", name="zps")
        if "z" not in SKIP:
            for e in grp:
                if not e["lead"]:
                    continue
                pj, pw = e["pj"], e["pw"]
                pr_rows = 22 * e["plen"]
                nc.tensor.matmul(
                    zps[:, e["oy"]:e["oy"] + pw],
                    lhsT=cxt[0:pr_rows, 128 * pj:128 * (pj + 1)],
                    rhs=r4t[0:pr_rows, e["poff"]:e["poff"] + pw],
                    start=True, stop=True, skip_group_check=True,
                    tile_position=(0, 0))
        return zps

    ebq = {}

    def issue_exp(i):
        gi = i % ng
        grp, ext, runs = groups[gi]
        zps = zq.pop(i)
        eb = ep.tile([128, ext], BF16, name="eb")
        if "exp" not in SKIP:
            spans = runs if ERF_RUNS else [(0, ext)]
            for (s, t) in spans:
                nc.scalar.activation(out=eb[:, s:t], in_=zps[:, s:t],
                                     func=AF.Exp)
        ebq[i] = eb

    def issue_mm(i):
        rb, gi = divmod(i, ng)
        r, b = divmod(rb, BPC)
        grp, ext, runs = groups[gi]
        eb = ebq.pop(i)
        ps = state[(r, b)][1]
        if "mm" not in SKIP:
            for e in grp:
                xw = e["x1"] - e["x0"]
                k = e["k"]
                for (h, p0, cw) in _mm_pieces(e["y0"], e["y1"]):
                    c0 = 128 * h + p0 - e["y0"]
                    nc.tensor.matmul(
                        ps[h][p0:p0 + cw, e["x0"]:e["x1"]],
                        lhsT=eb[:, e["oy"] + c0:e["oy"] + c0 + cw],
                        rhs=eb[:, e["ox"]:e["ox"] + xw],
                        start=False, stop=(k == last_h[h]),
                        skip_group_check=True,
                        tile_position=(0, p0))
        if gi == ng - 1 and "out" not in SKIP:
            if OUT_COPY == "psum":
                for h in range(2):
                    if halves[h] is None:
                        continue
                    r0, r1, c0, c1 = halves[h]
                    nc.sync.dma_start(
                        out[b, 128 * h + r0:128 * h + r1, c0:c1],
                        ps[h][r0:r1, c0:c1])
                return
            osb = osbp.tile([128, 2 * D], BF16 if OUT_BF16 else F32,
                            name="osb")
            for h in range(2):
                if halves[h] is None:
                    continue
                r0, r1, c0, c1 = halves[h]
                dst = osb[r0:r1, D * h + c0:D * h + c1]
                if OUT_COPY == "gpsimd":
                    nc.gpsimd.tensor_copy(dst, ps[h][r0:r1, c0:c1])
                elif OUT_COPY == "act":
                    nc.scalar.activation(out=dst, in_=ps[h][r0:r1, c0:c1],
                                         func=AF.Copy)
                else:
                    nc.vector.tensor_copy(dst, ps[h][r0:r1, c0:c1])
                nc.sync.dma_start(
                    out[b, 128 * h + r0:128 * h + r1, c0:c1], dst)

    # PE issue order per iteration: z(i+1) BEFORE mm(i-1), so the next
    # group's z (which gates ACT) isn't queued behind accumulation matmuls
    # that themselves wait on the current exp
    zq = {0: issue_z(0)}
    for i in range(nun):
        issue_exp(i)
        if i + 1 < nun:
            zq[i + 1] = issue_z(i + 1)
        if i >= 1:
            issue_mm(i - 1)
    issue_mm(nun - 1)


def _body(nc, pools, lb, zeros, r4t, coef, coefx, coefy, out, bins):
    entries = _emit_tiles(bins)
    for k, e in enumerate(entries):
        e["k"] = k
    # last matmul per PSUM half (for the stop flag)
    last_h = [None, None]
    for k, e in enumerate(entries):
        if e["y0"] < 128:
            last_h[0] = k
        if e["y1"] > 128:
            last_h[1] = k
    halves = _half_info(entries)
    coefp, zp, ep, ayp = pools["coef"], pools["z"], pools["e"], pools["ay"]
    osbp, psp = pools["osb"], pools["ps"]
    zx_eng = nc.gpsimd if ZX_ENG == "gpsimd" else nc.vector

    # hoist all batches' coef DMAs + PSUM zeroing to the body head
    per_batch = []
    for b in range(BPC):
        if QUAD_EXP and QUAD_FUSED:
            ct = cyt = None  # amp is folded into the x-side Exp bias rows
            cxt = coefp.tile(list(coefx[b].shape), BF16, tag="cx",
                             name="cxt")
            nc.sync.dma_start(cxt[:], coefx[b])
        elif QUAD_EXP:
            ct = None  # amp is folded into the x-side Exp bias rows
            cxt = coefp.tile([11, N], BF16, tag="cx", name="cxt")
            nc.sync.dma_start(cxt[:], coefx[b])
            cyt = coefp.tile([11, N], BF16, tag="cy", name="cyt")
            nc.sync.dma_start(cyt[:], coefy[b])
        elif Z_VIA_PE:
            # only the amp block of the packed f32 coefs is needed
            ct = coefp.tile([128, NT], F32, tag="coef", name="coef")
            nc.sync.dma_start(ct[:], coef[b][:, 3 * NT:4 * NT])
            cxt = coefp.tile([4, N], BF16, tag="cx", name="cxt")
            nc.sync.dma_start(cxt[:], coefx[b])
            cyt = coefp.tile([4, N], BF16, tag="cy", name="cyt")
            nc.sync.dma_start(cyt[:], coefy[b])
        else:
            ct = coefp.tile([128, 4 * NT], F32, tag="coef", name="coef")
            nc.sync.dma_start(ct[:], coef[b])
            cxt = cyt = None
        pst = psp.tile([128, 2 * D], F32, tag="ps", name="ps")
        ps = [pst[:, 0:D], pst[:, D:2 * D]]
        for h in range(2):
            if halves[h] is None:
                continue
            r0, r1, c0, c1 = halves[h]
            if ZERO_ENG == "pe":
                # start=True marks the region pending-zero and the zero
                # lhsT writes it; accumulating matmuls then use start=False
                for (_, p0, cw) in _mm_pieces(128 * h + r0, 128 * h + r1):
                    nc.tensor.matmul(
                        ps[h][p0:p0 + cw, c0:c1], lhsT=zeros[:, 0:cw],
                        rhs=lb[:, 0:c1 - c0], start=True, stop=False,
                        skip_group_check=True, tile_position=(0, p0))
            elif ZERO_ENG == "act":
                nc.scalar.activation(out=ps[h][r0:r1, c0:c1],
                                     in_=lb[r0:r1, 0:c1 - c0],
                                     func=AF.Copy, scale=0.0)
            else:
                nc.vector.memset(ps[h][r0:r1, c0:c1], 0.0)
        per_batch.append((ct, cxt, cyt, ps))

    for b in range(BPC):
        ct, cxt, cyt, ps = per_batch[b]
        if QUAD_EXP:
            spos = bnx = bny = amp = None
        elif Z_VIA_PE:
            spos = bnx = bny = None
            amp = ct[:, 0:NT]
        else:
            spos = ct[:, 0 * NT:1 * NT]
            bnx = ct[:, 1 * NT:2 * NT]
            bny = ct[:, 2 * NT:3 * NT]
            amp = ct[:, 3 * NT:4 * NT]

        for grp, ext, runs in _groups(entries):
            if QUAD_EXP:
                zps = pools["zps"].tile([128, ZPS_COLS], F32, tag="zps",
                                        name="zps")
                if "z" not in SKIP:
                    for e in grp:
                        a, yw = e["a"], e["y1"] - e["y0"]
                        xw = e["x1"] - e["x0"]
                        if QUAD_FUSED and QUAD_PAIR:
                            if e["lead"]:
                                pj, pw = e["pj"], e["pw"]
                                pr_rows = 22 * e["plen"]
                                nc.tensor.matmul(
                                    zps[:, e["oy"]:e["oy"] + pw],
                                    lhsT=cxt[0:pr_rows,
                                             128 * pj:128 * (pj + 1)],
                                    rhs=r4t[0:pr_rows,
                                            e["poff"]:e["poff"] + pw],
                                    start=True, stop=True,
                                    skip_group_check=True,
                                    tile_position=(0, 0))
                            continue
                        if QUAD_FUSED:
                            o = e["roff"]
                            nc.tensor.matmul(
                                zps[:, e["oy"]:e["oy"] + yw + xw],
                                lhsT=cxt[0:22, 128 * a:128 * (a + 1)],
                                rhs=r4t[0:22, o:o + yw + xw],
                                start=True, stop=True,
                                skip_group_check=True,
                                tile_position=(0, 0))
                            continue
                        rqx, rqy = r4t[e["bin"]]
                        nc.tensor.matmul(
                            zps[:, e["oy"]:e["oy"] + yw],
                            lhsT=cyt[0:11, 128 * a:128 * (a + 1)],
                            rhs=rqy[0:11, e["y0"]:e["y1"]],
                            start=True, stop=True, skip_group_check=True,
                            tile_position=(0, 0))
                        nc.tensor.matmul(
                            zps[:, e["ox"]:e["ox"] + xw],
                            lhsT=cxt[0:11, 128 * a:128 * (a + 1)],
                            rhs=rqx[0:11, e["x0"]:e["x1"]],
                            start=True, stop=True, skip_group_check=True,
                            tile_position=(0, 0))
                eb = ep.tile([128, ext], BF16, name="eb")
                if "exp" not in SKIP:
                    spans = runs if ERF_RUNS else [(0, ext)]
                    for (s, t) in spans:
                        nc.scalar.activation(out=eb[:, s:t], in_=zps[:, s:t],
                                             func=AF.Exp)
            elif Z_VIA_PE:
                zps = pools["zps"].tile([128, ZPS_COLS], F32, tag="zps",
                                        name="zps")
                if "z" not in SKIP:
                    for e in grp:
                        a, yw = e["a"], e["y1"] - e["y0"]
                        xw = e["x1"] - e["x0"]
                        nc.tensor.matmul(
                            zps[:, e["oy"]:e["oy"] + yw],
                            lhsT=cyt[0:4, 128 * a:128 * (a + 1)],
                            rhs=r4t[0:4, e["y0"]:e["y1"]],
                            start=True, stop=True, skip_group_check=True,
                            tile_position=(0, 0))
                        nc.tensor.matmul(
                            zps[:, e["ox"]:e["ox"] + xw],
                            lhsT=cxt[0:4, 128 * a:128 * (a + 1)],
                            rhs=r4t[0:4, e["x0"]:e["x1"]],
                            start=True, stop=True, skip_group_check=True,
                            tile_position=(0, 0))
                eb = ep.tile([128, ext], BF16, name="eb")
                if "exp" not in SKIP:
                    spans = runs if ERF_RUNS else [(0, ext)]
                    for (s, t) in spans:
                        nc.scalar.activation(out=eb[:, s:t], in_=zps[:, s:t],
                                             func=AF.Derivative_Erf)
            else:
                zb = zp.tile([128, ext], BF16, name="zb")
                if "z" not in SKIP:
                    for e in grp:
                        a, yw = e["a"], e["y1"] - e["y0"]
                        xw = e["x1"] - e["x0"]
                        nc.vector.tensor_scalar(
                            out=zb[:, e["oy"]:e["oy"] + yw],
                            in0=lb[:, e["y0"]:e["y1"]],
                            scalar1=spos[:, a:a + 1], scalar2=bny[:, a:a + 1],
                            op0=OP.mult, op1=OP.subtract)
                        zx_eng.tensor_scalar(
                            out=zb[:, e["ox"]:e["ox"] + xw],
                            in0=lb[:, e["x0"]:e["x1"]],
                            scalar1=spos[:, a:a + 1], scalar2=bnx[:, a:a + 1],
                            op0=OP.mult, op1=OP.subtract)
                eb = ep.tile([128, ext], BF16, name="eb")
                if "exp" not in SKIP:
                    nc.scalar.activation(out=eb[:], in_=zb[:],
                                         func=AF.Derivative_Erf)
            for e in grp:
                a, yw = e["a"], e["y1"] - e["y0"]
                xw = e["x1"] - e["x0"]
                if QUAD_EXP:
                    # amp is already folded into the x side by the Exp bias
                    if "mm" not in SKIP:
                        k = e["k"]
                        for (h, p0, cw) in _mm_pieces(e["y0"], e["y1"]):
                            c0 = 128 * h + p0 - e["y0"]
                            nc.tensor.matmul(
                                ps[h][p0:p0 + cw, e["x0"]:e["x1"]],
                                lhsT=eb[:, e["oy"] + c0:e["oy"] + c0 + cw],
                                rhs=eb[:, e["ox"]:e["ox"] + xw],
                                start=False, stop=(k == last_h[h]),
                                skip_group_check=True,
                                tile_position=(0, p0))
                    continue
                amp_on_y = not (AMP_MIN_SIDE and xw < yw)
                aw = yw if amp_on_y else xw
                src = e["oy"] if amp_on_y else e["ox"]
                ab = ayp.tile([128, 256], BF16, name="ay")
                if "amp" not in SKIP:
                    if AY_ENG == "scalar":
                        nc.scalar.activation(
                            out=ab[:, 0:aw], in_=eb[:, src:src + aw],
                            func=AF.Copy, scale=amp[:, a:a + 1])
                    else:
                        eng = nc.gpsimd if AY_ENG == "gpsimd" else nc.vector
                        eng.tensor_scalar(
                            out=ab[:, 0:aw], in0=eb[:, src:src + aw],
                            scalar1=amp[:, a:a + 1], scalar2=None,
                            op0=OP.mult)
                if "mm" not in SKIP:
                    k = e["k"]
                    for (h, p0, cw) in _mm_pieces(e["y0"], e["y1"]):
                        c0 = 128 * h + p0 - e["y0"]
                        lhsT = (ab[:, c0:c0 + cw] if amp_on_y
                                else eb[:, e["oy"] + c0:e["oy"] + c0 + cw])
                        rhs = (eb[:, e["ox"]:e["ox"] + xw] if amp_on_y
                               else ab[:, 0:xw])
                        nc.tensor.matmul(
                            ps[h][p0:p0 + cw, e["x0"]:e["x1"]],
                            lhsT=lhsT, rhs=rhs,
                            start=False, stop=(k == last_h[h]),
                            skip_group_check=True,
                            tile_position=(0, p0))
        if "out" not in SKIP:
            osb = osbp.tile([128, 2 * D], BF16 if OUT_BF16 else F32,
                            name="osb")
            for h in range(2):
                if halves[h] is None:
                    continue
                r0, r1, c0, c1 = halves[h]
                dst = osb[r0:r1, D * h + c0:D * h + c1]
                if OUT_COPY == "gpsimd":
                    nc.gpsimd.tensor_copy(dst, ps[h][r0:r1, c0:c1])
                elif OUT_COPY == "act" or (OUT_COPY == "act_dve" and h == 0):
                    nc.scalar.activation(out=dst, in_=ps[h][r0:r1, c0:c1],
                                         func=AF.Copy)
                else:
                    nc.vector.tensor_copy(dst, ps[h][r0:r1, c0:c1])
                nc.sync.dma_start(
                    out[b, 128 * h + r0:128 * h + r1, c0:c1], dst)


def build(bins, repeats=1):
    nc = bacc.Bacc("TRN2", target_bir_lowering=False, debug=False)
    pq = QUAD_EXP and QUAD_FUSED and QUAD_PAIR
    line_d = coef_d = None
    if not pq:
        line_d = nc.dram_tensor("line_bc", [128, D], BF16,
                                kind="ExternalInput")
        coef_d = nc.dram_tensor("coefs", [BPC, 128, 4 * NT], F32,
                                kind="ExternalInput")
    nb = len(bins)
    if QUAD_EXP and QUAD_FUSED and QUAD_PAIR:
        entries = _emit_tiles(bins)
        tot = max(sum((e["y1"] - e["y0"]) + (e["x1"] - e["x0"])
                      for e in entries), 1)
        npair = max(len(_zpairs(entries)), 1)
        coefx_d = nc.dram_tensor("coefq2",
                                 [BPC, 22 * ZFUSE_K, 128 * npair], BF16,
                                 kind="ExternalInput")
        coefy_d = None
        rq_d = nc.dram_tensor("rqf2", [22 * ZFUSE_K, tot], BF16,
                              kind="ExternalInput")
    elif QUAD_EXP and QUAD_FUSED:
        entries = _emit_tiles(bins)
        tot = max(sum((e["y1"] - e["y0"]) + (e["x1"] - e["x0"])
                      for e in entries), 1)
        coefx_d = nc.dram_tensor("coefq", [BPC, 22, N], BF16,
                                 kind="ExternalInput")
        coefy_d = None
        rq_d = nc.dram_tensor("rqf", [22, tot], BF16, kind="ExternalInput")
    elif QUAD_EXP:
        coefx_d = nc.dram_tensor("coefqx", [BPC, 11, N], BF16,
                                 kind="ExternalInput")
        coefy_d = nc.dram_tensor("coefqy", [BPC, 11, N], BF16,
                                 kind="ExternalInput")
        rq_d = nc.dram_tensor("rq", [max(nb, 1), 2, 11, D], BF16,
                              kind="ExternalInput")
    elif Z_VIA_PE:
        coefx_d = nc.dram_tensor("coefx", [BPC, 4, N], BF16,
                                 kind="ExternalInput")
        coefy_d = nc.dram_tensor("coefy", [BPC, 4, N], BF16,
                                 kind="ExternalInput")
        r4_d = nc.dram_tensor("r4", [4, D], BF16, kind="ExternalInput")
    out_d = nc.dram_tensor("out", [BPC, D, D], BF16 if OUT_BF16 else F32,
                           kind="ExternalOutput")
    with tile.TileContext(nc) as tc, ExitStack() as ctx:
        pools = {
            "const": ctx.enter_context(tc.tile_pool(name="const", bufs=1)),
            "coef": ctx.enter_context(tc.tile_pool(name="coef", bufs=BPC)),
            "z": ctx.enter_context(tc.tile_pool(name="z", bufs=3)),
            "e": ctx.enter_context(tc.tile_pool(name="e", bufs=3)),
            "ay": ctx.enter_context(tc.tile_pool(name="ay", bufs=8)),
            "osb": ctx.enter_context(tc.tile_pool(name="osb", bufs=4)),
        }
        if Z_VIA_PE or QUAD_EXP:
            pools["zps"] = ctx.enter_context(
                tc.tile_pool(name="zps", bufs=2, space="PSUM"))
            ps_bufs = max(1, (8 - 2 * (ZPS_COLS // 512)))
        else:
            ps_bufs = 4
        pools["ps"] = ctx.enter_context(
            tc.tile_pool(name="ps", bufs=min(4, ps_bufs), space="PSUM"))
        lb = zeros = None
        if not pq:
            lb = pools["const"].tile([128, D], BF16)
            nc.sync.dma_start(lb[:], line_d.ap())
            zeros = pools["const"].tile([128, 128], BF16)
            nc.vector.memset(zeros[:], 0.0)
        r4t = None
        if pq:
            # chunk the const DMA per exp group (pairs are bucket-ordered,
            # so each group's columns are contiguous): the first z-matmuls
            # wait only on their own group's chunk
            r4t = pools["const"].tile([22 * ZFUSE_K, tot], BF16, tag="rqf")
            entries_g = _groups(_emit_tiles(bins))
            off = 0
            for grp, ext, runs in entries_g:
                hi = max(e["poff"] + e["pw"] for e in grp)
                if hi > off:
                    nc.sync.dma_start(r4t[:, off:hi], rq_d.ap()[:, off:hi])
                    off = hi
            if off < tot:
                nc.sync.dma_start(r4t[:, off:tot], rq_d.ap()[:, off:tot])
        elif QUAD_EXP and QUAD_FUSED:
            r4t = pools["const"].tile([22, tot], BF16, tag="rqf")
            nc.sync.dma_start(r4t[:], rq_d.ap())
        elif QUAD_EXP:
            # per-(bin, side) rhs const tiles at partition base 0
            r4t = []
            for kb in range(nb):
                pair = []
                for side in range(2):
                    t = pools["const"].tile([11, D], BF16,
                                            tag=f"rq{kb}_{side}")
                    nc.sync.dma_start(t[:], rq_d.ap()[kb, side])
                    pair.append(t)
                r4t.append(pair)
        elif Z_VIA_PE:
            r4t = pools["const"].tile([4, D], BF16)
            nc.sync.dma_start(r4t[:], r4_d.ap())
        if Z_VIA_PE or QUAD_EXP:
            if not ERF_RUNS:
                # pre-write both zps pool bufs so group-spanning erf' reads
                # of bank-padding gaps never see cold PSUM
                for _ in range(2):
                    t = pools["zps"].tile([128, ZPS_COLS], F32, tag="zps",
                                          name="zps")
                    nc.vector.memset(t[:], 0.0)
        via_pe = Z_VIA_PE or QUAD_EXP
        if QUAD_EXP and QUAD_FUSED and QUAD_PAIR:
            _body_pq(nc, pools, None, r4t, coefx_d.ap(), out_d.ap(),
                     bins, repeats=repeats)
        else:
            for _ in range(repeats):
                _body(nc, pools, lb[:], zeros[:],
                      (r4t if QUAD_EXP
                       else (r4t[:] if r4t is not None else None)),
                      coef_d.ap(),
                      coefx_d.ap() if via_pe else None,
                      coefy_d.ap() if (via_pe and coefy_d is not None)
                      else None,
                      out_d.ap(), bins)
    nc.compile()
    return nc


def kernel(line_coords, rot_mats, centers, sigmas, amplitudes):
    line_coords = np.ascontiguousarray(np.asarray(line_coords, np.float32))
    rot_mats = np.ascontiguousarray(np.asarray(rot_mats, np.float32))
    centers = np.ascontiguousarray(np.asarray(centers, np.float32))
    sigmas = np.ascontiguousarray(np.asarray(sigmas, np.float32))
    amplitudes = np.ascontiguousarray(np.asarray(amplitudes, np.float32))

    bins, in_maps = make_in_maps(line_coords, rot_mats, centers, sigmas,
                                 amplitudes)
    nc = build(bins)
    res = run_bass_kernel_spmd(nc, in_maps, list(range(NCORES)))
    out = np.concatenate([np.asarray(res.results[c]["out"])
                          for c in range(NCORES)], axis=0)
    return np.ascontiguousarray(out.astype(np.float32))

